# revision 32
# baseline (speedup 1.0000x reference)
"""Additive (Bahdanau-style) attention on 8 TRN2 NeuronCores.

reference:
    q = queries @ Wq                      (B,Tq,H)
    k = keys @ Wk                         (B,Tk,H)
    scores[b,i,j] = sum_h wv[h] * tanh(q[b,i,h] + k[b,j,h])
    out = softmax(scores) @ values        (B,Tq,Dv)

The (B,Tq,Tk,H) tanh intermediate is replaced by a separable sine
expansion fitted under the N(0,2) distribution of q+k:

    tanh(s) ~= sum_m c_m sin(w_m s)
    tanh(a+b) ~= sum_m c_m [sin(w_m a)cos(w_m b) + cos(w_m a)sin(w_m b)]

so scores become one accumulated matmul with contraction dim 2*M*H.

Range reduction for ACT's Sin (valid on [-pi, pi] only) is done in phase
units on DVE:  f0 = (q * w/2pi) floormod 1  in [0,1), f1 = (f0 + 0.25)
floormod 1, then ACT evaluates sin(2pi*f - pi) = -sin(2pi*f); the minus
signs cancel in the q*k products.  Phases ride in fp16 (2^-11 phase
quantization is ~1e-3 in the scores, inside budget).

Sharding: data-parallel over batch B=8, one batch element per core.
k-side is loaded and projected first so the ACT sin pipeline (the
bottleneck engine) starts as early as possible.
"""

import numpy as np
import ml_dtypes

import concourse.bass as bass
import concourse.tile as tile
from concourse import bacc, mybir
from concourse.bass_utils import run_bass_kernel_spmd
from bass_rust import add_dep_helper


def _chain(insts, reason):
    """Force scheduling order within one engine's queue."""
    for a, b in zip(insts[1:], insts[:-1]):
        add_dep_helper(a.ins, b.ins, sync=False, reason=reason)

B, TQ, TK = 8, 256, 256
DQ, DK, DV, H = 512, 512, 512, 256

M = 3
OMEGA = np.array([0.4597, 1.4288, 2.5691])
TWO_PI = 2.0 * np.pi
KBITS = 12
MASK = (1 << KBITS) - 1

F32 = mybir.dt.float32
BF16 = mybir.dt.bfloat16
FP16 = mybir.dt.float16
I16 = mybir.dt.int16
I32 = mybir.dt.int32
AF = mybir.ActivationFunctionType
ALU = mybir.AluOpType


def _fit_coeffs():
    x = np.linspace(0.0, 9.0, 6001)
    w = np.exp(-x * x / 4.0) + 1e-3
    A = np.sin(np.outer(x, OMEGA))
    sw = np.sqrt(w)[:, None]
    c, *_ = np.linalg.lstsq(A * sw, np.tanh(x) * sw[:, 0], rcond=None)
    return c.astype(np.float64)

COEF = _fit_coeffs()

_CACHE = {}


def _build_graph():
    nc = bacc.Bacc("TRN2", target_bir_lowering=False, debug=False,
                   enable_asserts=False, num_devices=B)

    ins = {}
    for nm in ("kin", "qin"):
        ins[nm] = nc.dram_tensor(nm, (128, 8, 256), FP16,
                                 kind="ExternalInput").ap()
    ins["vals"] = nc.dram_tensor("vals", (128, 2, DV), BF16,
                                 kind="ExternalInput").ap()
    ins["cwv"] = nc.dram_tensor("cwv", (128, M, 2), F32,
                                kind="ExternalInput").ap()
    out = nc.dram_tensor("out", (128, 2, DV), FP16, kind="ExternalOutput").ap()

    with tile.TileContext(nc) as tc:
        with tc.tile_pool(name="sb", bufs=1) as sb, \
             tc.tile_pool(name="pk", bufs=1, space="PSUM") as pk, \
             tc.tile_pool(name="pq", bufs=1, space="PSUM") as pq, \
             tc.tile_pool(name="psc", bufs=1, space="PSUM") as psc, \
             tc.tile_pool(name="pwarm", bufs=1, space="PSUM") as pwarm, \
             tc.tile_pool(name="pout", bufs=2, space="PSUM") as pout:
            _body(nc, tc, sb, pk, pq, psc, pwarm, pwarm, pout, ins, out)
    nc.compile()
    return nc


def _body(nc, tc, sb, pk, pq, psc, pwarm, psm, pout, ins, out):
    # ---- SBUF tiles ----
    kin_sb = sb.tile([128, 8, 256], FP16)       # [wk d0..3 | ksT d0..3]
    qin_sb = sb.tile([128, 8, 256], FP16)
    wk_sb, ksT_sb = kin_sb[:, 0:4, :], kin_sb[:, 4:8, :]
    wq_sb, qsT_sb = qin_sb[:, 0:4, :], qin_sb[:, 4:8, :]
    vals_bf = sb.tile([128, 2, DV], BF16)       # [k%128, khalf, v]
    cwv_sb = sb.tile([128, M, 2], F32)          # [h%128, m, jhalf]
    junk = sb.tile([128, 128], BF16)            # ones for the rowsum matmul
    junk2 = sb.tile([128, 128], BF16)           # uninitialized HAM-warmup fuel
    qT = sb.tile([128, 2 * TQ], FP16)
    yk = sb.tile([128, M, 2, 2 * TK], I16)      # round(k*w*2^12/2pi) (+2^10 cos)
    yq = sb.tile([128, M, 2, 2 * TQ], I16)
    phk = sb.tile([128, M, 2, 2 * TK], I16)     # y & 0xFFF
    phq = sb.tile([128, M, 2, 2 * TQ], I16)
    sk = sb.tile([128, M, 2, 2 * TK], BF16)     # -sin(2pi*ph)
    sq = sb.tile([128, M, 2, 2 * TQ], BF16)
    sqs = sb.tile([128, M, 2, 2 * TQ], BF16)    # amp * sq
    attn = sb.tile([128, 2, TQ], BF16)          # [k%128, khalf, qi] = exp(sT)
    rcp = sb.tile([128, 2], F32)                # 1/rowsum per qi (a-half)
    o = sb.tile([128, 2, DV], FP16)

    # ---- input DMA across the 3 DMA-capable queues; k-side first ----
    nc.vector.memset(junk[:], 1.0)
    nc.gpsimd.memset(junk2[:], 1.0)
    negpi = sb.tile([128, 1], F32)
    nc.vector.memset(negpi[:], float(-np.pi))
    nc.sync.dma_start(kin_sb[:, 0:3, :], ins["kin"][:, 0:3, :])
    nc.gpsimd.dma_start(kin_sb[:, 3:6, :], ins["kin"][:, 3:6, :])
    nc.scalar.dma_start(kin_sb[:, 6:8, :], ins["kin"][:, 6:8, :])
    nc.sync.dma_start(qin_sb[:, 0:3, :], ins["qin"][:, 0:3, :])
    nc.gpsimd.dma_start(qin_sb[:, 3:6, :], ins["qin"][:, 3:6, :])
    nc.scalar.dma_start(qin_sb[:, 6:8, :], ins["qin"][:, 6:8, :])
    nc.gpsimd.dma_start(cwv_sb[:], ins["cwv"])
    nc.sync.dma_start(vals_bf[:, 0:1, :], ins["vals"][:, 0:1, :])
    nc.scalar.dma_start(vals_bf[:, 1:2, :], ins["vals"][:, 1:2, :])

    # HAM warmup on PE during the DMA wait (uninitialized data: PSUM junk
    # is never read; the tile is re-cleared by the rowsum's start=True)
    ps_warm = pwarm.tile([128, 128], F32, name="ps_warm", tag="ps_warm")
    warm = sb.tile([128, 1], F32)
    nc.scalar.activation(warm[:], negpi[:], AF.Sin, bias=0.0, scale=0.1)
    pe_order = []
    for _ in range(32):
        pe_order.append(nc.tensor.matmul(ps_warm[:], junk2[:], junk2[:],
                                         start=True, stop=True))

    # ---- projections: xT[h, (j,i)] = sum_d W[d, j*128+h] * xsT[d, i] ----
    ps_k = pk.tile([128, 2, TK], F32, name="ps_k", tag="ps_k")
    ps_q = pq.tile([128, 2, TQ], F32, name="ps_q", tag="ps_q")
    for (w_sb, x_sb, ps, n) in ((wk_sb, ksT_sb, ps_k, TK),
                                (wq_sb, qsT_sb, ps_q, TQ)):
        for j in range(2):
            for d in range(4):
                pe_order.append(nc.tensor.matmul(
                    ps[:, j, :], w_sb[:, d, bass.ts(j, 128)], x_sb[:, d, :],
                    start=(d == 0), stop=(d == 3)))
    # bridge junk to keep HAM warm between projections and score matmuls;
    # gated on sin completions (below) so they fill the gap, not the front
    bridges = []
    for _ in range(16):
        mm = nc.tensor.matmul(ps_warm[:], junk2[:], junk2[:],
                              start=True, stop=True)
        bridges.append(mm)
        pe_order.append(mm)

    # ---- phases: int16 fixed point, two's-complement floormod.
    # k-side y0 = round(sc*kT) comes straight from PSUM on ACT (Copy w/
    # scale, int16 out); q-side y0 on DVE from an fp16 SBUF copy.
    qcp = [nc.vector.tensor_copy(qT[:, bass.ts(j, TQ)], ps_q[:, j, :])
           for j in range(2)]
    act_order = []
    phase_insts = {}
    sc0 = float(OMEGA[0] * (1 << KBITS) / TWO_PI)
    for j in range(2):
        act_order.append(nc.scalar.activation(
            yk[:, 0, 0, bass.ts(j, TK)], ps_k[:, j, :], AF.Copy,
            bias=0.0, scale=sc0))
    for m in range(1, M):
        sc = float(OMEGA[m] * (1 << KBITS) / TWO_PI)
        act_order.append(nc.scalar.activation(
            yk[:, m, 0, :], ps_k[:, :, :], AF.Copy, bias=0.0, scale=sc))
    for side, (y, ph) in enumerate(((yk, phk), (yq, phq))):
        for m in range(M):
            sc = float(OMEGA[m] * (1 << KBITS) / TWO_PI)
            ops = []
            if side == 1:
                ops.append(nc.vector.tensor_scalar(
                    out=y[:, m, 0, :], in0=qT[:],
                    scalar1=sc, scalar2=None, op0=ALU.mult))
            ops.append(nc.vector.tensor_scalar(
                out=y[:, m, 1, :], in0=y[:, m, 0, :],
                scalar1=(1 << (KBITS - 2)), scalar2=None, op0=ALU.add))
            ops.append(nc.vector.tensor_scalar(
                out=ph[:, m, :, :].bitcast(I32), in0=y[:, m, :, :].bitcast(I32),
                scalar1=(MASK | (MASK << 16)), scalar2=None,
                op0=ALU.bitwise_and))
            phase_insts[(side, m)] = ops
    # DVE queue order: k m0 first (unblocks the first sin), then q copy/m0
    dve_order = [*phase_insts[(0, 0)], *qcp, *phase_insts[(1, 0)],
                 *phase_insts[(0, 1)], *phase_insts[(1, 1)]]

    # ---- sins on ACT (the bottleneck: k/q alternating, m ascending) ----
    SC = float(TWO_PI / (1 << KBITS))
    for m in range(M):
        act_order.append(nc.scalar.activation(
            sk[:, m, :, :], phk[:, m, :, :], AF.Sin,
            bias=negpi[:], scale=SC))
        act_order.append(nc.scalar.activation(
            sq[:, m, :, :], phq[:, m, :, :], AF.Sin,
            bias=negpi[:], scale=SC))
    _chain(act_order, "act sin order")

    # ---- amplitudes (c_m * wv_h) on DVE, interleaved with late phases ----
    amp_insts = {}
    for m in range(M):
        amp_insts[m] = [nc.vector.tensor_scalar_mul(
            out=sqs[:, m, :, bass.ts(j, TQ)],
            in0=sq[:, m, :, bass.ts(j, TQ)],
            scalar1=cwv_sb[:, m, j:j + 1]) for j in range(2)]
    dve_order += [*amp_insts[0], *phase_insts[(0, 2)], *phase_insts[(1, 2)],
                  *amp_insts[1], *amp_insts[2]]
    _chain(dve_order, "dve pipeline order")
    # bridges fire as the sins complete, keeping PE busy through the gap
    for i, mm in enumerate(bridges):
        add_dep_helper(mm.ins, act_order[4 + min(i // 8, 1)].ins, sync=True,
                       reason="bridge gated on sin")

    # ---- score matmuls: scoresT += trig_k^T (amp*trig_q) ----
    ps_a = [psc.tile([128, TQ], F32, name=f"ps_sc{a}", tag=f"ps_sc{a}", bufs=1)
            for a in range(2)]
    for m in range(M):
        for j in range(2):
            for kh in range(2):
                for (qq, kq) in ((0, 1), (1, 0)):
                    pe_order.append(nc.tensor.matmul(
                        ps_a[kh][:],
                        sk[:, m, kq, bass.ds(j * TK + kh * 128, 128)],
                        sqs[:, m, qq, bass.ts(j, TQ)],
                        start=(m == 0 and j == 0 and (qq, kq) == (0, 1)),
                        stop=(m == M - 1 and j == 1 and (qq, kq) == (1, 0))))

    # ---- softmax (deferred normalization, on scoresT) ----
    for kh in range(2):
        act_order.append(nc.scalar.activation(
            attn[:, kh, :], ps_a[kh][:], AF.Exp, bias=0.0, scale=1.0))
    for a in range(2):
        sm = psm.tile([128, 1], F32, name=f"sm{a}", tag="ps_warm", bufs=1)
        for kh in range(2):
            pe_order.append(nc.tensor.matmul(
                sm[:], attn[:, kh, bass.ts(a, 128)], junk[:, 0:1],
                start=(kh == 0), stop=(kh == 1)))
        nc.vector.reciprocal(rcp[:, a:a + 1], sm[:])

    # ---- out = attnT.T @ values, scaled by 1/rowsum (scale on ACT) ----
    for a in range(2):
        po = pout.tile([128, DV], F32)
        for kh in range(2):
            pe_order.append(nc.tensor.matmul(
                po[:], attn[:, kh, bass.ts(a, 128)], vals_bf[:, kh, :],
                start=(kh == 0), stop=(kh == 1)))
        if a == 0:
            dve_order.append(nc.vector.tensor_scalar_mul(
                out=o[:, a, :], in0=po[:], scalar1=rcp[:, a:a + 1]))
        else:
            act_order.append(nc.scalar.activation(
                o[:, a, :], po[:], AF.Copy, bias=0.0, scale=rcp[:, a:a + 1]))
    _chain(act_order, "act order")
    _chain(pe_order, "pe order")
    nc.sync.dma_start(out[:, 0, 0:256], o[:, 0, 0:256])
    nc.gpsimd.dma_start(out[:, 0, 256:512], o[:, 0, 256:512])
    nc.sync.dma_start(out[:, 1, 0:256], o[:, 1, 0:256])
    nc.gpsimd.dma_start(out[:, 1, 256:512], o[:, 1, 256:512])


def _shuffle(x):
    """(512, n) -> (128, 4, n) with [d%128, dchunk, i]."""
    return np.ascontiguousarray(x.reshape(4, 128, x.shape[1]).transpose(1, 0, 2))


def kernel(queries, keys, values, Wq, Wk, wv, _trace=False):
    if "g" not in _CACHE:
        _CACHE["g"] = _build_graph()
    nc = _CACHE["g"]

    cwv = (COEF[None, :, None] *
           wv.astype(np.float64).reshape(2, 128).T[:, None, :]).astype(np.float32)
    wqs = _shuffle(Wq.astype(np.float16))
    wks = _shuffle(Wk.astype(np.float16))
    base = {"cwv": cwv}
    in_maps = []
    for b in range(B):
        m = dict(base)
        m["qin"] = np.ascontiguousarray(np.concatenate(
            [wqs, _shuffle(queries[b].T.astype(np.float16))], axis=1))
        m["kin"] = np.ascontiguousarray(np.concatenate(
            [wks, _shuffle(keys[b].T.astype(np.float16))], axis=1))
        v = values[b].astype(ml_dtypes.bfloat16)
        m["vals"] = np.ascontiguousarray(v.reshape(2, 128, DV).transpose(1, 0, 2))
        in_maps.append(m)
    kw = {"trace": True, "trace_cores": [0]} if _trace else {}
    res = run_bass_kernel_spmd(nc, in_maps, core_ids=list(range(B)), **kw)
    _CACHE["last"] = res
    return np.stack(
        [np.ascontiguousarray(
            res.results[b]["out"].transpose(1, 0, 2).reshape(TQ, DV))
         .astype(np.float32) for b in range(B)], axis=0)
